# revision 2
# baseline (speedup 1.0000x reference)
"""Trainium2 Bass kernel for nn_AttentionModel (greedy pointer-attention decode).

v3: row-compacted sparse streaming decode with LAZY swap-with-last.

Per step t, L = 200-t slots are streamed; exactly one slot (the stale slot
s(t), masked with an additive -2^30) holds the row removed at step t-1.  The
physical swap (last slot -> s(t)) runs off the argmax critical path, so the
next step's streaming DMA overlaps the current step's compute.  q comes from
a precomputed Qp = (emb+pref) @ W_step[256:] table indexed by ORIGINAL node
id (posmap_d gather -> qp_d gather), so kq_d's reader set stays argmax-free.

Engine split (n-major compat layout keeps every GPSIMD pattern contiguous):
  mult1/red1 : alternate DVE / GPSIMD by chunk
  mult2      : GPSIMD      red2: DVE (strided transposed read)
  mult3      : GPSIMD      red3: alternate
  exp/tanh-exp/ln : ACT (exp/ln table set only; tanh computed via exp)
  argmax on raw dots (tanh monotonic); log-softmax + local_scatter output
  entirely off the critical path.
"""
import numpy as np

import concourse.bass as bass
from concourse import bacc
import concourse.tile as tile
from concourse import mybir
from concourse.bass import IndirectOffsetOnAxis
from concourse.bass_utils import run_bass_kernel_spmd
from concourse.tile_rust import add_dep_helper


def _dep(later, earlier, reason):
    """later waits on earlier (explicit edge for indirect-DMA hazards the
    tile dependency tracker cannot derive from dynamic access patterns)."""
    if later is not None and earlier is not None:
        add_dep_helper(later.ins, earlier.ins, sync=True, reason=reason)

dt = mybir.dt
F32 = dt.float32
I32 = dt.int32
I16 = dt.int16
U32 = dt.uint32
AX = mybir.AxisListType
OP = mybir.AluOpType
ACTF = mybir.ActivationFunctionType

B, N, D, H = 1024, 200, 256, 8
NCORES = 8
BS = B // NCORES                # 128 items per core
T = N - 1                       # 199 decode steps
START = 24
NEG = -1e9
MSK = -float(2 ** 30)           # exact-in-fp32 mask addend
MSKI16HI = -12672               # high int16 of fp32(-2^30) bit pattern
ROWC = 3 * D                    # kq row: gK|gV|lK
ISD32 = float(np.float32(1.0 / np.sqrt(32.0)))

CS_KV = 24                      # rows per K / V stream chunk (1KB lines)
CS_LK = 16                      # rows per lK stream chunk (1KB lines)
KV_BUFS = 3
LK_BUFS = 2
NCH_MAX = (N + CS_KV - 1) // CS_KV

T_STEPS = T


def _build(t_steps=None):
    t_steps = T_STEPS if t_steps is None else t_steps
    nc = bacc.Bacc("TRN2", target_bir_lowering=False, debug=False)

    emb_in = nc.dram_tensor("embeddings", [BS, N, D], F32, kind="ExternalInput").ap()
    pref_in = nc.dram_tensor("pref_embed", [D], F32, kind="ExternalInput").ap()
    wnode_in = nc.dram_tensor("W_node", [D, 3 * D], F32, kind="ExternalInput").ap()
    wfix_in = nc.dram_tensor("W_fixed", [D, D], F32, kind="ExternalInput").ap()
    wstep_in = nc.dram_tensor("W_step", [2 * D, D], F32, kind="ExternalInput").ap()
    wout_in = nc.dram_tensor("W_out", [D, D], F32, kind="ExternalInput").ap()

    out = nc.dram_tensor("log_p", [BS, T * N], F32, kind="ExternalOutput").ap()

    kq_d = nc.dram_tensor("kq_d", [BS * N, ROWC], F32).ap()       # slot-indexed
    qp_d = nc.dram_tensor("qp_d", [BS * N, D], F32).ap()          # ORIG-pos-indexed
    idx_d = nc.dram_tensor("idx_d", [BS * N, 1], I32).ap()        # slot -> out pair
    pos_d = nc.dram_tensor("pos_d", [BS * N, 1], I32).ap()        # slot -> orig pos

    kq_bnc = kq_d.rearrange("(b n) c -> b n c", b=BS)
    kq_sec = kq_d.rearrange("r (s c) -> (r s) c", s=3)    # 256-col sections
    idx_bn = idx_d.rearrange("(b n) o -> b (n o)", b=BS)
    pos_bn = pos_d.rearrange("(b n) o -> b (n o)", b=BS)

    with tile.TileContext(nc) as tc:
        with (
            tc.tile_pool(name="wpool", bufs=1) as wpool,
            tc.tile_pool(name="kvs", bufs=KV_BUFS) as kvs,
            tc.tile_pool(name="lks", bufs=LK_BUFS) as lks,
            tc.tile_pool(name="cmp", bufs=1) as cmppool,
            tc.tile_pool(name="work", bufs=2) as work,
            tc.tile_pool(name="lpp", bufs=2) as lpp,
            tc.tile_pool(name="swp", bufs=2) as swp,
            tc.tile_pool(name="psum", bufs=2, space="PSUM") as psp,
            tc.tile_pool(name="psum1", bufs=2, space="PSUM") as psp1,
        ):
            # ---------------- persistent setup ----------------
            wbig = wpool.tile([128, 2, 1024], F32)   # [W_node | W_step[256:]]
            nc.sync.dma_start(wbig[:, 0, 0:768], wnode_in[0:128, :])
            nc.sync.dma_start(wbig[:, 1, 0:768], wnode_in[128:256, :])
            nc.sync.dma_start(wbig[:, 0, 768:1024], wstep_in[256:384, :])
            nc.sync.dma_start(wbig[:, 1, 768:1024], wstep_in[384:512, :])
            wo_sb = wpool.tile([128, 2, D], F32)
            nc.sync.dma_start(wo_sb[:, 0, :], wout_in[0:128, :])
            nc.sync.dma_start(wo_sb[:, 1, :], wout_in[128:256, :])
            wf_sb = wpool.tile([128, 2, D], F32)
            nc.sync.dma_start(wf_sb[:, 0, :], wfix_in[0:128, :])
            nc.sync.dma_start(wf_sb[:, 1, :], wfix_in[128:256, :])
            ws1_sb = wpool.tile([128, 2, D], F32)
            nc.sync.dma_start(ws1_sb[:, 0, :], wstep_in[0:128, :])
            nc.sync.dma_start(ws1_sb[:, 1, :], wstep_in[128:256, :])

            pref_sb = wpool.tile([128, D], F32)
            nc.sync.dma_start(
                pref_sb[:],
                pref_in.rearrange("(o f) -> o f", o=1).broadcast_to([128, D]),
            )

            ident = wpool.tile([128, 128], F32)
            io_c = wpool.tile([128, 128], I32)
            nc.gpsimd.iota(io_c[:], pattern=[[1, 128]], channel_multiplier=0)
            io_r = wpool.tile([128, 1], I32)
            nc.gpsimd.iota(io_r[:], pattern=[[0, 1]], channel_multiplier=1)
            id_i = wpool.tile([128, 128], I32)
            nc.vector.tensor_tensor(id_i[:], io_c[:], io_r[:].broadcast_to([128, 128]), op=OP.is_equal)
            nc.vector.tensor_copy(ident[:], id_i[:])

            iota_row = wpool.tile([128, 1], I32)     # b*200
            nc.gpsimd.iota(iota_row[:], pattern=[[0, 1]], channel_multiplier=N)

            fixed2 = wpool.tile([128, D], F32)
            q_sb = wpool.tile([128, D], F32)

            # idx pairs (2p, 2p+1) int16; pos_d = iota
            iota_n = wpool.tile([128, N], I32)
            nc.gpsimd.iota(iota_n[:], pattern=[[1, N]], channel_multiplier=0)
            idx0 = wpool.tile([128, N, 2], I16)
            e2j = wpool.tile([128, N], I32)
            nc.vector.tensor_tensor(e2j[:], iota_n[:], iota_n[:], op=OP.add)
            nc.vector.tensor_copy(idx0[:, :, 0], e2j[:])
            nc.vector.tensor_scalar(e2j[:], e2j[:], 1, None, op0=OP.add)
            nc.vector.tensor_copy(idx0[:, :, 1], e2j[:])
            nc.sync.dma_start(idx_bn[:, :], idx0[:].rearrange("p n t -> p (n t)").bitcast(I32))
            nc.sync.dma_start(pos_bn[:, :], iota_n[:])

            negpair = wpool.tile([128, 2], I16)      # fp32(-2^30) as int16 pair
            nc.vector.memset(negpair[:, 0:1], 0)
            nc.vector.memset(negpair[:, 1:2], MSKI16HI)

            # lazy-swap state (read old value each step, then overwritten)
            s_f = wpool.tile([128, 1], F32)
            nc.vector.memset(s_f[:], float(START))
            offs_stale = wpool.tile([128, 1], I32)
            nc.vector.tensor_scalar(offs_stale[:], iota_row[:], START, None, op0=OP.add)

            amask0 = wpool.tile([128, 208], F32)
            nc.vector.memset(amask0[:], 0.0)
            nc.vector.memset(amask0[:, START:START + 1], MSK)

            # ---------------- precompute: kq + qp rows ----------------
            emb_rows = emb_in.rearrange("b n c -> (b n) c")
            ROWT = BS * N // 128

            def pre_body(rt):
                erow = work.tile([128, D], F32, tag="erow")
                nc.sync.dma_start(erow[:], emb_rows[bass.ds(rt * 128, 128), :])
                e2 = work.tile([128, D], F32, tag="e2")
                nc.vector.tensor_tensor(e2[:], erow[:], pref_sb[:], op=OP.add)
                e2T = work.tile([128, 2, 128], F32, tag="e2T")
                for ci in range(2):
                    tp = psp1.tile([128, 128], F32, tag="tp")
                    nc.tensor.transpose(tp[:], e2[:, ci * 128:(ci + 1) * 128], ident[:])
                    nc.scalar.copy(e2T[:, ci, :], tp[:])
                kv = work.tile([128, 1024], F32, tag="kv")
                for fh in range(2):
                    pm = psp.tile([128, 512], F32, tag="ps")
                    nc.tensor.matmul(pm[:], e2T[:, 0, :], wbig[:, 0, fh * 512:(fh + 1) * 512], start=True, stop=False)
                    nc.tensor.matmul(pm[:], e2T[:, 1, :], wbig[:, 1, fh * 512:(fh + 1) * 512], start=False, stop=True)
                    if fh == 0:
                        nc.scalar.copy(kv[:, 0:512], pm[:])
                    else:
                        nc.vector.tensor_copy(kv[:, 512:1024], pm[:])
                nc.sync.dma_start(kq_d[bass.ds(rt * 128, 128), :], kv[:, 0:ROWC])
                nc.sync.dma_start(qp_d[bass.ds(rt * 128, 128), :], kv[:, ROWC:1024])

            tc.For_i_unrolled(0, ROWT, 1, pre_body, max_unroll=2)

            # ---------------- fixed2 ----------------
            macc = work.tile([128, D], F32, tag="macc")
            for c in range((N + CS_LK - 1) // CS_LK):
                c0 = c * CS_LK
                cs = min(CS_LK, N - c0)
                ech = lks.tile([128, CS_LK, D], F32, tag="lk")
                nc.sync.dma_start(ech[:, 0:cs, :], emb_in[:, c0:c0 + cs, :])
                part = work.tile([128, D], F32, tag="mpart")
                nc.vector.tensor_reduce(part[:], ech[:, 0:cs, :].transpose([0, 2, 1]), axis=AX.X, op=OP.add)
                if c == 0:
                    nc.vector.tensor_copy(macc[:], part[:])
                else:
                    nc.vector.tensor_tensor(macc[:], macc[:], part[:], op=OP.add)
            nc.vector.tensor_scalar(macc[:], macc[:], 1.0 / N, None, op0=OP.mult)
            nc.vector.tensor_tensor(macc[:], macc[:], pref_sb[:], op=OP.add)
            first2 = work.tile([128, D], F32, tag="first2")
            nc.sync.dma_start(first2[:], emb_in[:, START, :])
            nc.vector.tensor_tensor(first2[:], first2[:], pref_sb[:], op=OP.add)

            fT = work.tile([128, 2, 128], F32, tag="fT")
            mT = work.tile([128, 2, 128], F32, tag="mT")
            for ci in range(2):
                tp = psp1.tile([128, 128], F32, tag="tp")
                nc.tensor.transpose(tp[:], macc[:, ci * 128:(ci + 1) * 128], ident[:])
                nc.scalar.copy(mT[:, ci, :], tp[:])
                tp2 = psp1.tile([128, 128], F32, tag="tp")
                nc.tensor.transpose(tp2[:], first2[:, ci * 128:(ci + 1) * 128], ident[:])
                nc.scalar.copy(fT[:, ci, :], tp2[:])
            pf = psp.tile([128, 512], F32, tag="ps")
            nc.tensor.matmul(pf[:, 0:D], mT[:, 0, :], wf_sb[:, 0, :], start=True, stop=False)
            nc.tensor.matmul(pf[:, 0:D], mT[:, 1, :], wf_sb[:, 1, :], start=False, stop=False)
            nc.tensor.matmul(pf[:, 0:D], fT[:, 0, :], ws1_sb[:, 0, :], start=False, stop=False)
            nc.tensor.matmul(pf[:, 0:D], fT[:, 1, :], ws1_sb[:, 1, :], start=False, stop=True)
            nc.scalar.copy(fixed2[:], pf[:, 0:D])

            # q0 = fixed2 + Qp[orig 24]
            qp0 = work.tile([128, D], F32, tag="qp")
            nc.sync.dma_start(qp0[:], qp_d.rearrange("(b n) c -> b n c", b=BS)[:, START, :])
            nc.vector.tensor_tensor(q_sb[:], qp0[:], fixed2[:], op=OP.add)

            # ---------------- decode steps ----------------
            amask_cur = amask0
            prev_w = {"kv": [], "lk": None, "idx": None, "pos": None}

            def step_body(t):
                nonlocal amask_cur
                L = N - t                      # streamed slots (incl 1 stale)
                lastp = (t == t_steps - 1) or (L == 2)
                LW = max(L, 8)
                kv_reads = []
                lk_reads = []

                # --- early prefetches (depend only on step t-1's swap) ---
                sw = None
                if not lastp:
                    sw = swp.tile([128, ROWC], F32, tag="sw")
                    swi = nc.sync.dma_start(sw[:], kq_bnc[:, L - 1, :])
                    for w in prev_w["kv"]:
                        _dep(swi, w, "sw read after prev swap-write")
                    _dep(swi, prev_w["lk"], "sw read after prev swap-write lk")
                    kv_reads.append(swi)
                    lk_reads.append(swi)
                    plast = work.tile([128, 1], I32, tag="plast")
                    pli = nc.sync.dma_start(plast[:], pos_bn[:, L - 1:L])
                    _dep(pli, prev_w["pos"], "plast read after prev pos-scatter")
                idxt = lpp.tile([128, N], I32, tag="idx")
                ixi = nc.sync.dma_start(idxt[:, 0:L], idx_bn[:, 0:L])
                _dep(ixi, prev_w["idx"], "idx read after prev idx-scatter")
                idx16v = idxt[:, 0:L].bitcast(I16)

                compat = cmppool.tile([128, N, H], F32, tag="compat")   # n-major
                gvp = cmppool.tile([128, NCH_MAX, D], F32, tag="gvp")
                logits = lpp.tile([128, 208], F32, tag="logits")

                qb = q_sb[:].rearrange("p (o f) -> p o f", o=1)
                amv = amask_cur[:]

                # --- K-pass: compat dots (torch operation order) ---
                nch = (L + CS_KV - 1) // CS_KV
                for c in range(nch):
                    c0 = c * CS_KV
                    cs = min(CS_KV, L - c0)
                    kvt = kvs.tile([128, CS_KV, D], F32, tag="kv")
                    dq = nc.sync if c % 2 == 0 else nc.scalar
                    di = dq.dma_start(kvt[:, 0:cs, :], kq_bnc[:, c0:c0 + cs, 0:256])
                    for w in prev_w["kv"]:
                        _dep(di, w, "K stream after prev swap-write")
                    kv_reads.append(di)
                    kpart = kvt[:, 0:cs, :]
                    nc.gpsimd.tensor_tensor(kpart, kpart, qb.broadcast_to([128, cs, D]), op=OP.mult)
                    nc.vector.tensor_reduce(compat[:, c0:c0 + cs, :],
                                            kpart.rearrange("p n (h e) -> p n h e", h=H),
                                            axis=AX.X, op=OP.add)
                # scale, stale mask, per-head max-subtract softmax (matches ref)
                cL = compat[:, 0:L, :]
                nc.vector.tensor_scalar(cL, cL, ISD32, None, op0=OP.mult)
                nc.gpsimd.tensor_tensor(
                    cL, cL,
                    amv[:, 0:L].rearrange("p (n o) -> p n o", o=1).broadcast_to([128, L, H]),
                    op=OP.add)
                mh = work.tile([128, H], F32, tag="mh")
                nc.vector.tensor_reduce(mh[:], cL.transpose([0, 2, 1]), axis=AX.X, op=OP.max)
                nc.gpsimd.tensor_tensor(
                    cL, cL,
                    mh[:].rearrange("p (o h) -> p o h", o=1).broadcast_to([128, L, H]),
                    op=OP.subtract)
                nc.scalar.activation(cL, cL, ACTF.Exp)
                sh = work.tile([128, H], F32, tag="sh")
                nc.vector.tensor_reduce(sh[:], cL.transpose([0, 2, 1]), axis=AX.X, op=OP.add)
                rh = work.tile([128, H], F32, tag="rh")
                nc.vector.reciprocal(rh[:], sh[:])
                nc.gpsimd.tensor_tensor(
                    cL, cL,
                    rh[:].rearrange("p (o h) -> p o h", o=1).broadcast_to([128, L, H]),
                    op=OP.mult)

                # --- V-pass: glimpse = attn @ V ---
                for c in range(nch):
                    c0 = c * CS_KV
                    cs = min(CS_KV, L - c0)
                    vt = kvs.tile([128, CS_KV, D], F32, tag="kv")
                    dq = nc.sync if c % 2 == 0 else nc.scalar
                    di = dq.dma_start(vt[:, 0:cs, :], kq_bnc[:, c0:c0 + cs, 256:512])
                    for w in prev_w["kv"]:
                        _dep(di, w, "V stream after prev swap-write")
                    kv_reads.append(di)
                    av = compat[:, c0:c0 + cs, :].rearrange("p n (h o) -> p n h o", o=1) \
                        .broadcast_to([128, cs, H, 32])
                    nc.gpsimd.tensor_tensor(vt[:, 0:cs, :].rearrange("p n (h e) -> p n h e", h=H),
                                            vt[:, 0:cs, :].rearrange("p n (h e) -> p n h e", h=H),
                                            av, op=OP.mult)
                    nc.vector.tensor_reduce(gvp[:, c, :], vt[:, 0:cs, :].transpose([0, 2, 1]),
                                            axis=AX.X, op=OP.add)
                if nch > 1:
                    glim2 = work.tile([128, D], F32, tag="glim2")
                    nc.vector.tensor_reduce(glim2[:], gvp[:, 0:nch, :].transpose([0, 2, 1]),
                                            axis=AX.X, op=OP.add)
                    glimv = glim2[:]
                else:
                    glimv = gvp[:, 0, :]

                # g = glimpse @ W_out
                gT = work.tile([128, 2, 128], F32, tag="gT")
                for ci in range(2):
                    tp = psp1.tile([128, 128], F32, tag="tp")
                    nc.tensor.transpose(tp[:], glimv[ci * 128:(ci + 1) * 128] if False else glimv[:, ci * 128:(ci + 1) * 128], ident[:])
                    nc.scalar.copy(gT[:, ci, :], tp[:])
                pg = psp.tile([128, 512], F32, tag="ps")
                nc.tensor.matmul(pg[:, 0:D], gT[:, 0, :], wo_sb[:, 0, :], start=True, stop=False)
                nc.tensor.matmul(pg[:, 0:D], gT[:, 1, :], wo_sb[:, 1, :], start=False, stop=True)
                g_sb = work.tile([128, D], F32, tag="g_sb")
                nc.scalar.copy(g_sb[:], pg[:, 0:D])
                gb = g_sb[:].rearrange("p (o f) -> p o f", o=1)

                # --- pass3: logits ---
                if LW > L:
                    nc.vector.memset(logits[:, L:LW], NEG)
                nch3 = (L + CS_LK - 1) // CS_LK
                for c in range(nch3):
                    c0 = c * CS_LK
                    cs = min(CS_LK, L - c0)
                    lkt = lks.tile([128, CS_LK, D], F32, tag="lk")
                    lkq = nc.scalar if c % 2 == 0 else nc.sync
                    di = lkq.dma_start(lkt[:, 0:cs, :], kq_bnc[:, c0:c0 + cs, 512:768])
                    _dep(di, prev_w["lk"], "lK stream after prev swap-write")
                    lk_reads.append(di)
                    nc.gpsimd.tensor_tensor(lkt[:, 0:cs, :], lkt[:, 0:cs, :],
                                            gb.broadcast_to([128, cs, D]), op=OP.mult)
                    nc.vector.tensor_reduce(logits[:, c0:c0 + cs], lkt[:, 0:cs, :],
                                            axis=AX.X, op=OP.add)
                nc.gpsimd.tensor_tensor(logits[:, 0:L], logits[:, 0:L], amv[:, 0:L], op=OP.add)

                # --- argmax + lazy-swap bookkeeping ---
                if not lastp:
                    mx8 = work.tile([128, 8], F32, tag="mx8")
                    nc.vector.max(mx8[:], logits[:, 0:LW])
                    ix8 = work.tile([128, 8], U32, tag="ix8")
                    nc.vector.max_index(ix8[:], mx8[:], logits[:, 0:LW])
                    j32 = work.tile([128, 1], I32, tag="j32")
                    nc.vector.tensor_copy(j32[:], ix8[:, 0:1])
                    jf = work.tile([128, 1], F32, tag="jf")
                    nc.vector.tensor_copy(jf[:], j32[:])

                    # physical swap of stale slot (uses OLD offs_stale), by
                    # 256-col section so next KV stream only waits on KV reads
                    o3 = work.tile([128, 1], I32, tag="o3")
                    nc.vector.tensor_tensor(o3[:], offs_stale[:], offs_stale[:], op=OP.add)
                    nc.vector.tensor_tensor(o3[:], o3[:], offs_stale[:], op=OP.add)
                    wsec = []
                    for si in range(3):
                        osi = work.tile([128, 1], I32, tag=f"osec{si}")
                        nc.vector.tensor_scalar(osi[:], o3[:], si, None, op0=OP.add)
                        wi = nc.gpsimd.indirect_dma_start(
                            out=kq_sec, out_offset=IndirectOffsetOnAxis(ap=osi[:], axis=0),
                            in_=sw[:, si * 256:(si + 1) * 256], in_offset=None)
                        for r in (kv_reads if si < 2 else lk_reads):
                            _dep(wi, r, "swap-write after this step's section reads")
                        wsec.append(wi)
                    widx = nc.gpsimd.indirect_dma_start(
                        out=idx_d, out_offset=IndirectOffsetOnAxis(ap=offs_stale[:], axis=0),
                        in_=idxt[:, L - 1:L], in_offset=None)
                    _dep(widx, ixi, "idx-scatter after idx read")
                    wpos = nc.gpsimd.indirect_dma_start(
                        out=pos_d, out_offset=IndirectOffsetOnAxis(ap=offs_stale[:], axis=0),
                        in_=plast[:], in_offset=None)
                    _dep(wpos, pli, "pos-scatter after plast read")

                    # q for next step: orig pos of selected slot -> Qp row
                    offs_j = work.tile([128, 1], I32, tag="offs_j")
                    nc.vector.tensor_tensor(offs_j[:], iota_row[:], j32[:], op=OP.add)
                    pstar = work.tile([128, 1], I32, tag="pstar")
                    pgi = nc.gpsimd.indirect_dma_start(
                        out=pstar[:], out_offset=None,
                        in_=pos_d, in_offset=IndirectOffsetOnAxis(ap=offs_j[:], axis=0))
                    _dep(pgi, prev_w["pos"], "pstar gather after prev pos-scatter")
                    prev_w["kv"] = wsec[0:2]
                    prev_w["lk"] = wsec[2]
                    prev_w["idx"], prev_w["pos"] = widx, wpos
                    offs_qp = work.tile([128, 1], I32, tag="offs_qp")
                    nc.vector.tensor_tensor(offs_qp[:], iota_row[:], pstar[:], op=OP.add)
                    qp = work.tile([128, D], F32, tag="qp")
                    nc.gpsimd.indirect_dma_start(
                        out=qp[:], out_offset=None,
                        in_=qp_d, in_offset=IndirectOffsetOnAxis(ap=offs_qp[:], axis=0))
                    nc.vector.tensor_tensor(q_sb[:], qp[:], fixed2[:], op=OP.add)

                    # s_next = (j* == L-1) ? s_old : j*
                    cf = work.tile([128, 1], F32, tag="cf")
                    nc.vector.tensor_scalar(cf[:], jf[:], float(L - 1), None, op0=OP.is_equal)
                    dsj = work.tile([128, 1], F32, tag="dsj")
                    nc.vector.tensor_tensor(dsj[:], s_f[:], jf[:], op=OP.subtract)
                    nc.vector.tensor_tensor(dsj[:], dsj[:], cf[:], op=OP.mult)
                    sn = work.tile([128, 1], F32, tag="sn")
                    nc.vector.tensor_tensor(sn[:], jf[:], dsj[:], op=OP.add)
                    nc.vector.tensor_copy(s_f[:], sn[:])
                    sni = work.tile([128, 1], I32, tag="sni")
                    nc.vector.tensor_copy(sni[:], sn[:])
                    nc.vector.tensor_tensor(offs_stale[:], iota_row[:], sni[:], op=OP.add)

                    # amask for next step: -2^30 one-hot at s_next
                    s2i = work.tile([128, 1], I32, tag="s2i")
                    nc.vector.tensor_tensor(s2i[:], sni[:], sni[:], op=OP.add)
                    sidx = work.tile([128, 2], I16, tag="sidx")
                    nc.vector.tensor_copy(sidx[:, 0:1], s2i[:])
                    nc.vector.tensor_scalar(s2i[:], s2i[:], 1, None, op0=OP.add)
                    nc.vector.tensor_copy(sidx[:, 1:2], s2i[:])
                    amn = lpp.tile([128, 208], F32, tag="amask")
                    nc.gpsimd.local_scatter(
                        out_ap=amn[:].bitcast(I16),
                        data_ap=negpair[:],
                        idxs_ap=sidx[:],
                        channels=128, num_elems=416, num_idxs=2)
                    amask_cur = amn

                # --- output path (off critical path) ---
                # t10 = 10*tanh(dot/16) = 10 - 20/(exp(dot/8)+1)
                e1 = lpp.tile([128, 208], F32, tag="e1")
                nc.scalar.activation(e1[:, 0:LW], logits[:, 0:LW], ACTF.Exp, scale=0.125)
                nc.vector.tensor_scalar(e1[:, 0:LW], e1[:, 0:LW], 1.0, None, op0=OP.add)
                nc.vector.reciprocal(e1[:, 0:LW], e1[:, 0:LW])
                t10 = lpp.tile([128, 208], F32, tag="t10")
                nc.vector.tensor_scalar(t10[:, 0:LW], e1[:, 0:LW], -20.0, 10.0,
                                        op0=OP.mult, op1=OP.add)
                if LW > L:
                    nc.vector.memset(t10[:, L:LW], NEG)
                nc.gpsimd.tensor_tensor(t10[:, 0:L], t10[:, 0:L], amv[:, 0:L], op=OP.add)
                s1 = work.tile([128, 1], F32, tag="s1")
                nc.scalar.activation(e1[:, 0:LW], t10[:, 0:LW], ACTF.Exp, accum_out=s1[:])
                ls = work.tile([128, 1], F32, tag="ls")
                nc.scalar.activation(ls[:], s1[:], ACTF.Ln)
                ls1 = work.tile([128, 1], F32, tag="ls1")
                nc.vector.tensor_scalar(ls1[:], ls[:], 1.0, None, op0=OP.add)
                lpm1 = e1
                nc.vector.tensor_tensor(lpm1[:, 0:L], t10[:, 0:L],
                                        ls1[:].broadcast_to([128, L]), op=OP.subtract)
                # zero the stale slot's value so its scatter lands as "visited"
                zm = lpp.tile([128, 208], F32, tag="zm")
                nc.vector.tensor_scalar(zm[:, 0:L], amv[:, 0:L], float(2.0 ** -30), 1.0,
                                        op0=OP.mult, op1=OP.add)
                nc.gpsimd.tensor_tensor(lpm1[:, 0:L], lpm1[:, 0:L], zm[:, 0:L], op=OP.mult)
                lp_full = lpp.tile([128, N], F32, tag="lpf")
                nc.gpsimd.local_scatter(
                    out_ap=lp_full[:].bitcast(I16),
                    data_ap=lpm1[:, 0:L].bitcast(I16),
                    idxs_ap=idx16v,
                    channels=128, num_elems=2 * N, num_idxs=2 * L)
                m1 = lpp.tile([128, N], F32, tag="m1")
                nc.vector.tensor_scalar(m1[:], lp_full[:], 0.0, NEG,
                                        op0=OP.is_equal, op1=OP.mult)
                nc.vector.tensor_tensor(m1[:], m1[:], lp_full[:], op=OP.add)
                nc.vector.tensor_scalar(m1[:], m1[:], 1.0, None, op0=OP.add)
                nc.sync.dma_start(out[:, bass.ds(t * N, N)], m1[:])

            for t in range(t_steps):
                step_body(t)

    nc.compile()
    return nc


_CACHE = {}


def kernel(**inputs) -> np.ndarray:
    if "nc" not in _CACHE:
        _CACHE["nc"] = _build()
    nc = _CACHE["nc"]

    emb = np.ascontiguousarray(np.asarray(inputs["embeddings"], np.float32))
    shared = {
        "pref_embed": np.asarray(inputs["pref_embed"], np.float32),
        "W_node": np.asarray(inputs["W_node"], np.float32),
        "W_fixed": np.asarray(inputs["W_fixed"], np.float32),
        "W_step": np.asarray(inputs["W_step"], np.float32),
        "W_out": np.asarray(inputs["W_out"], np.float32),
    }
    in_maps = []
    for i in range(NCORES):
        m = {"embeddings": emb[i * BS:(i + 1) * BS]}
        m.update(shared)
        in_maps.append(m)

    res = run_bass_kernel_spmd(nc, in_maps, list(range(NCORES)))
    outs = [res.results[i]["log_p"].reshape(BS, T, N) for i in range(NCORES)]
    return np.concatenate(outs, axis=0)


if __name__ == "__main__":
    z = np.load("inputs.npz")
    inp = {k: z[k] for k in z.files}
    o = kernel(**inp)
    print("kernel output", o.shape, o.dtype)
    np.save("kernel_out.npy", o)
